# revision 14
# baseline (speedup 1.0000x reference)
"""Trainium2 Bass kernel for single-head causal attention.

Problem: x:[4,2048,768], Wq/Wk/Wv:[768,768] (torch-Linear layout, y = x @ W.T),
out = causal_softmax(q k^T / sqrt(768)) @ v, all float32.

Sharding (8 NeuronCores, no collectives):
  - core pair (2b, 2b+1) handles batch b.
  - per batch, the 16 query tiles of 128 rows are split between the pair as
    {0,3,4,7,8,11,12,15} and {1,2,5,6,9,10,13,14}. Sorted by causal length
    those are {1,4,5,8,9,12,13,16} and {2,3,6,7,10,11,14,15} key-tiles, so
    both sides fit the same static per-slot key budget {2,4,...,16}: the one
    SPMD graph processes 8 query tiles whose key ranges are padded by at most
    one 128-tile (+6% flops) and the pad/diagonal is handled by a host-
    provided additive mask over the last two key blocks of every slot.
  - the K projection is folded away algebraically: scores = q k^T =
    x_q (Wq^T Wk) x_kv^T, with M = Wq^T Wk precomputed on the host from the
    weights. The device computes qk = x_q @ M and contracts scores directly
    against x^T, which is already resident in SBUF. Similarly the V
    projection runs after the attention-weighted sum: out = (probs @ x) Wv^T.
  - the per-slot work is software-pipelined as
    ... scores(s+1) | out-projection(s) | attn-V(s+1) ... so the last slot's
    exp/copy latency hides under the previous slot's matmuls.
"""

import math
import os
import sys

import numpy as np

if not any(os.path.isdir(os.path.join(p, "concourse")) for p in sys.path):
    sys.path.insert(0, "/opt/trn_rl_repo")

import concourse.bass as bass  # noqa: E402
import concourse.mybir as mybir  # noqa: E402
from concourse import bacc, tile  # noqa: E402
from concourse.bass_utils import run_bass_kernel_spmd  # noqa: E402
from concourse.masks import make_identity  # noqa: E402

import ml_dtypes  # noqa: E402

B, S, D = 4, 2048, 768
P = 128
NT = S // P          # 16 key tiles per batch
DC = D // P          # 6 contraction chunks
NSLOT = 8            # query tiles per core
QROWS = NSLOT * P    # 1024 query rows per core
N_CORES = 8
SCALE = 1.0 / math.sqrt(D)

SIDE_A = [0, 3, 4, 7, 8, 11, 12, 15]   # causal lengths 1,4,5,8,9,12,13,16
SIDE_B = [1, 2, 5, 6, 9, 10, 13, 14]   # causal lengths 2,3,6,7,10,11,14,15
CAP = [2, 4, 6, 8, 10, 12, 14, 16]     # static key tiles per slot (>= real)

BF16 = ml_dtypes.bfloat16

_NC = None


def build():
    """Build + compile the single SPMD graph run by all 8 cores."""
    f32 = mybir.dt.float32
    bf16 = mybir.dt.bfloat16

    nc = bacc.Bacc("TRN2", target_bir_lowering=False, debug=False,
                   num_devices=N_CORES)

    # inputs come pre-packed as [P, chunk, width] (host layout transform)
    xq_d = nc.dram_tensor("xqT", [P, 2, DC, 512], bf16,
                          kind="ExternalInput").ap()
    xkv_d = nc.dram_tensor("xkvT", [P, DC, S], bf16,
                           kind="ExternalInput").ap()
    xkvr_d = nc.dram_tensor("xkvR", [P, NT, D], bf16,
                            kind="ExternalInput").ap()
    m_d = nc.dram_tensor("mT", [P, 3, DC, 256], bf16,
                         kind="ExternalInput").ap()
    wv_d = nc.dram_tensor("wvT", [P, DC, D], bf16, kind="ExternalInput").ap()
    mask_d = nc.dram_tensor("mask", [P, NSLOT, 256], bf16,
                            kind="ExternalInput").ap()
    out_d = nc.dram_tensor("out", [QROWS, D], f32, kind="ExternalOutput").ap()

    with tile.TileContext(nc) as tc:
        with (
            tc.tile_pool(name="const", bufs=1) as const,
            tc.tile_pool(name="probs", bufs=5) as probs_pool,
            tc.tile_pool(name="lsums", bufs=5) as lsum_pool,
            tc.tile_pool(name="pt", bufs=3) as pt_pool,
            tc.tile_pool(name="osb", bufs=2) as osb_pool,
            tc.tile_pool(name="small", bufs=2) as small,
            tc.tile_pool(name="ps_s", bufs=2, space="PSUM") as ps_s,
            tc.tile_pool(name="ps_tr", bufs=2, space="PSUM") as ps_tr,
            tc.tile_pool(name="ps_o", bufs=2, space="PSUM") as ps_o,
        ):
            # ---- persistent SBUF tensors, split in halves of 3 d-chunks
            # each so input DMA (12KB+ descriptors) overlaps the projections
            HC = DC // 2
            m_p = [const.tile([P, DC, 256], bf16, tag=f"mp{i}",
                              name=f"mp{i}") for i in range(3)]
            wv_h = [const.tile([P, HC, D], bf16, tag=f"wvh{h}", name=f"wvh{h}")
                    for h in range(2)]
            xq_g = [const.tile([P, DC, 512], bf16, tag=f"xqg{g}",
                               name=f"xqg{g}") for g in range(2)]
            xkv_h = [const.tile([P, HC, S], bf16, tag=f"xkvh{h}",
                                name=f"xkvh{h}") for h in range(2)]

            def chunk(tiles, dc):
                return tiles[dc // HC][:, dc % HC, :]

            wv_c = [chunk(wv_h, c) for c in range(DC)]
            xkv_c = [chunk(xkv_h, c) for c in range(DC)]
            mask_sb = const.tile([P, NSLOT, 256], bf16, tag="mask")
            ident = const.tile([P, P], bf16, tag="ident")
            qt_sb = const.tile([P, DC, QROWS], bf16, tag="qt")
            xv_h = [const.tile([P, NT // 2, D], bf16, tag=f"xvh{h}",
                                name=f"xvh{h}") for h in range(2)]

            # priority-ordered input DMAs: qk-projection operands first, then
            # the x^T chunks the score matmuls stream from
            nc.sync.dma_start(out=xq_g[0][:, :, :], in_=xq_d[:, 0, :, :])
            for i in range(3):
                nc.sync.dma_start(out=m_p[i][:, :, :], in_=m_d[:, i, :, :])
            nc.sync.dma_start(out=xkv_h[0][:, :, :], in_=xkv_d[:, 0:HC, :])
            nc.sync.dma_start(out=xq_g[1][:, :, :], in_=xq_d[:, 1, :, :])
            nc.sync.dma_start(out=xkv_h[1][:, :, :], in_=xkv_d[:, HC:DC, :])
            make_identity(nc, ident[:, :])

            # HAM warm-up: keep the PE busy while the first inputs stream in
            # so the real matmuls run at 2.4GHz from the start.
            warm = ps_tr.tile([P, P], f32, tag="tr", name="warm")
            for _ in range(52):
                nc.tensor.matmul(warm[:, :], ident[:, :], ident[:, :],
                                 start=True, stop=True)

            # ---- qkT[e,q] = (x_q @ M)^T projection (group-major: starts on
            # the first DMAs; group 0 covers slots 0-3)
            for g in range(QROWS // 512):
                for oc in range(DC):
                    ps = ps_s.tile([P, 512], f32, tag="mm512")
                    for dc in range(DC):
                        nc.tensor.matmul(
                            ps[:, :],
                            m_p[oc // 2][:, dc,
                                         (oc % 2) * P:(oc % 2 + 1) * P],
                            xq_g[g][:, dc, :],
                            start=(dc == 0), stop=(dc == DC - 1))
                    nc.scalar.copy(qt_sb[:, oc, g * 512:(g + 1) * 512],
                                   ps[:, :])
                if g == 0:
                    nc.sync.dma_start(out=mask_sb[:, :, :],
                                      in_=mask_d[:, :, :])
                    nc.sync.dma_start(
                        out=xv_h[0][:, :, :],
                        in_=xkvr_d[:, 0:NT // 2, :])
                    for h in range(2):
                        nc.sync.dma_start(out=wv_h[h][:, :, :],
                                          in_=wv_d[:, h * HC:(h + 1) * HC, :])
                    nc.sync.dma_start(
                        out=xv_h[1][:, :, :],
                        in_=xkvr_d[:, NT // 2:NT, :])

            # ---- attention, software-pipelined per 128-row query slot
            slot_bufs = {}

            def emit_scores(s):
                L = CAP[s]
                nk = L * P
                nkg = (nk + 511) // 512
                probs = probs_pool.tile([P, S], bf16, tag="probs",
                                        name=f"probs{s}")
                lsum = lsum_pool.tile([P, 4], f32, tag="lsum",
                                      name=f"lsum{s}")
                slot_bufs[s] = (probs, lsum)
                for g in range(nkg):
                    kw = min(512, nk - g * 512)
                    ps = ps_s.tile([P, 512], f32, tag="mm512",
                                   name=f"sps{s}_{g}")
                    for oc in range(DC):
                        nc.tensor.matmul(
                            ps[:, :kw],
                            qt_sb[:, oc, s * P:(s + 1) * P],
                            xkv_c[oc][:, g * 512:g * 512 + kw],
                            start=(oc == 0), stop=(oc == DC - 1))
                    mstart = (L - 2) * P   # masked region: last two blocks
                    if g * 512 <= mstart < g * 512 + kw:
                        off = mstart - g * 512
                        nc.vector.tensor_add(ps[:, off:off + 256],
                                             ps[:, off:off + 256],
                                             mask_sb[:, s, :])
                    nc.scalar.activation(
                        probs[:, g * 512:g * 512 + kw], ps[:, :kw],
                        mybir.ActivationFunctionType.Exp,
                        scale=SCALE, accum_out=lsum[:, g:g + 1])

            def emit_rest_av(s):
                """Transpose probability tiles and accumulate Y = probs @ x."""
                L = CAP[s]
                probs, lsum = slot_bufs[s]
                out_ps = ps_o.tile([P, D], f32, tag="mmout", name=f"ops{s}")
                slot_bufs[s] = (probs, lsum, out_ps)
                for kg in range((L + 3) // 4):     # 4 transposes per bank,
                    kn = min(4, L - kg * 4)        # one wide copy per group
                    tp = ps_tr.tile([P, 512], bf16, tag="tr", name=f"tp{s}")
                    for j in range(kn):
                        kt = kg * 4 + j
                        nc.tensor.transpose(tp[:, j * P:(j + 1) * P],
                                            probs[:, kt * P:(kt + 1) * P],
                                            ident[:, :])
                    pT = pt_pool.tile([P, 512], bf16, tag="pT", name=f"pT{s}")
                    nc.vector.tensor_copy(pT[:, 0:kn * P], tp[:, 0:kn * P])
                    for j in range(kn):
                        kt = kg * 4 + j
                        xv = xv_h[kt // (NT // 2)][:, kt % (NT // 2), :]
                        nc.tensor.matmul(out_ps[:, 0:512],
                                         pT[:, j * P:(j + 1) * P],
                                         xv[:, 0:512],
                                         start=(kt == 0), stop=(kt == L - 1))
                        nc.tensor.matmul(out_ps[:, 512:D],
                                         pT[:, j * P:(j + 1) * P],
                                         xv[:, 512:D],
                                         start=(kt == 0), stop=(kt == L - 1))

            def emit_rest_out(s):
                """Scale Y by 1/l, transpose, apply Wv: out = (Y/l) @ Wv^T."""
                L = CAP[s]
                nkg = (L * P + 511) // 512
                probs, lsum, out_ps = slot_bufs.pop(s)
                rinv = small.tile([P, 1], f32, tag="rinv", name=f"rinv{s}")
                if nkg > 1:
                    rsum = small.tile([P, 1], f32, tag="rsum",
                                      name=f"rsum{s}")
                    nc.vector.tensor_reduce(rsum[:, :], lsum[:, 0:nkg],
                                            axis=mybir.AxisListType.X,
                                            op=mybir.AluOpType.add)
                    nc.vector.reciprocal(rinv[:, :], rsum[:, :])
                else:
                    nc.vector.reciprocal(rinv[:, :], lsum[:, 0:1])

                # normalized ybar = Y/l while copying PSUM->SBUF (bf16), the
                # two halves on different engines
                y_lo = osb_pool.tile([P, 512], bf16, tag="ylo", name=f"ylo{s}")
                y_hi = osb_pool.tile([P, 256], bf16, tag="yhi", name=f"yhi{s}")
                nc.scalar.activation(y_lo[:, :], out_ps[:, 0:512],
                                     mybir.ActivationFunctionType.Copy,
                                     scale=rinv[:, :])
                nc.vector.tensor_scalar_mul(y_hi[:, :], out_ps[:, 512:D],
                                            rinv[:, :])
                ytT = pt_pool.tile([P, D], bf16, tag="ytT", name=f"ytT{s}")
                for kg in range(2):
                    kn = 4 if kg == 0 else 2
                    tp = ps_tr.tile([P, 512], bf16, tag="tr", name=f"ytp{s}")
                    for j in range(kn):
                        dt = kg * 4 + j
                        ysrc = (y_lo[:, dt * P:(dt + 1) * P] if dt < 4 else
                                y_hi[:, (dt - 4) * P:(dt - 3) * P])
                        nc.tensor.transpose(tp[:, j * P:(j + 1) * P],
                                            ysrc, ident[:, :])
                    nc.vector.tensor_copy(
                        ytT[:, kg * 512:kg * 512 + kn * P],
                        tp[:, 0:kn * P])
                out2_ps = ps_o.tile([P, D], f32, tag="mmout", name=f"o2ps{s}")
                for dc in range(DC):
                    nc.tensor.matmul(out2_ps[:, 0:512],
                                     ytT[:, dc * P:(dc + 1) * P],
                                     wv_c[dc][:, 0:512],
                                     start=(dc == 0), stop=(dc == DC - 1))
                for dc in range(DC):
                    nc.tensor.matmul(out2_ps[:, 512:D],
                                     ytT[:, dc * P:(dc + 1) * P],
                                     wv_c[dc][:, 512:D],
                                     start=(dc == 0), stop=(dc == DC - 1))
                # out2 is already normalized; copy out on both engines and DMA
                out_sb = osb_pool.tile([P, D], f32, tag="osb", name=f"osb{s}")
                nc.scalar.copy(out_sb[:, 0:512], out2_ps[:, 0:512])
                nc.sync.dma_start(out=out_d[s * P:(s + 1) * P, 0:512],
                                  in_=out_sb[:, 0:512])
                nc.vector.tensor_copy(out_sb[:, 512:D], out2_ps[:, 512:D])
                nc.sync.dma_start(out=out_d[s * P:(s + 1) * P, 512:D],
                                  in_=out_sb[:, 512:D])

            # slots run in causal-length order; the two largest slots'
            # scores are emitted back-to-back so the last slot's exp chain
            # hides under the second-to-last slot's matmuls
            for s in range(NSLOT - 2):
                emit_scores(s)
                emit_rest_av(s)
                emit_rest_out(s)
            emit_scores(NSLOT - 2)
            emit_scores(NSLOT - 1)
            emit_rest_av(NSLOT - 2)
            emit_rest_out(NSLOT - 2)
            emit_rest_av(NSLOT - 1)
            emit_rest_out(NSLOT - 1)

    nc.compile()
    return nc


def _pack(matT):
    """[D, W] (transposed operand) -> [P, DC, W] chunk layout, bf16."""
    d, w = matT.shape
    return np.ascontiguousarray(
        matT.reshape(d // P, P, w).transpose(1, 0, 2)).astype(BF16)


def shard_inputs(x, Wq, Wk, Wv):
    x = np.asarray(x, dtype=np.float32)
    # fold the K projection into the Q side: scores = x_q (Wq^T Wk) x_kv^T
    m = np.asarray(Wq, np.float32).T @ np.asarray(Wk, np.float32)
    mT = _pack(m)                                        # [P, DC, D]
    mT = np.ascontiguousarray(                           # [P, 3, DC, 256]
        mT.reshape(P, DC, 3, 256).transpose(0, 2, 1, 3))
    wvT = _pack(np.asarray(Wv, np.float32).T)
    in_maps = []
    for c in range(N_CORES):
        b, side = divmod(c, 2)
        qtiles = SIDE_A if side == 0 else SIDE_B
        xb = x[b]                                    # [S, D]
        xkvT = _pack(np.ascontiguousarray(xb.T))
        xkvR = np.ascontiguousarray(                     # [P, NT, D] row-major
            xb.astype(BF16).reshape(NT, P, D).transpose(1, 0, 2))
        xq = np.concatenate([xb[t * P:(t + 1) * P] for t in qtiles], axis=0)
        xqT = _pack(np.ascontiguousarray(xq.T))          # [P, DC, QROWS]
        xqT = np.ascontiguousarray(                      # [P, 2, DC, 512]
            xqT.reshape(P, DC, 2, 512).transpose(0, 2, 1, 3))
        mask = np.empty((NSLOT, P, 256), np.float32)
        for s, t in enumerate(qtiles):
            L = CAP[s]
            qidx = t * P + np.arange(P)[:, None]
            kidx = (L - 2) * P + np.arange(256)[None, :]
            mask[s] = np.where(kidx <= qidx, 0.0, -1e30).astype(np.float32)
        # mask dram layout [P, NSLOT, 256]
        mask = np.ascontiguousarray(mask.transpose(1, 0, 2)).astype(BF16)
        in_maps.append({"xqT": xqT, "xkvT": xkvT, "xkvR": xkvR, "mT": mT,
                        "wvT": wvT, "mask": mask})
    return in_maps


def unshard(results):
    out = np.empty((B, S, D), np.float32)
    for c in range(N_CORES):
        b, side = divmod(c, 2)
        qtiles = SIDE_A if side == 0 else SIDE_B
        oc = results[c]["out"]
        for s, t in enumerate(qtiles):
            out[b, t * P:(t + 1) * P] = oc[s * P:(s + 1) * P]
    return out


def run(inputs, trace=False, trace_cores=None):
    """Run on hardware; returns (output, BassKernelResults)."""
    global _NC
    if _NC is None:
        _NC = build()
    in_maps = shard_inputs(inputs["x"], inputs["Wq"], inputs["Wk"],
                           inputs["Wv"])
    res = run_bass_kernel_spmd(_NC, in_maps, core_ids=list(range(N_CORES)),
                               trace=trace, trace_cores=trace_cores)
    return unshard(res.results), res


def kernel(x, Wq, Wk, Wv):
    out, _ = run({"x": x, "Wq": Wq, "Wk": Wk, "Wv": Wv})
    return out


# revision 15
# speedup vs baseline: 1.0253x; 1.0253x over previous
"""Trainium2 Bass kernel for single-head causal attention.

Problem: x:[4,2048,768], Wq/Wk/Wv:[768,768] (torch-Linear layout, y = x @ W.T),
out = causal_softmax(q k^T / sqrt(768)) @ v, all float32.

Sharding (8 NeuronCores, no collectives):
  - core pair (2b, 2b+1) handles batch b.
  - per batch, the 16 query tiles of 128 rows are split between the pair as
    {0,3,4,7,8,11,12,15} and {1,2,5,6,9,10,13,14}. Sorted by causal length
    those are {1,4,5,8,9,12,13,16} and {2,3,6,7,10,11,14,15} key-tiles, so
    both sides fit the same static per-slot key budget {2,4,...,16}: the one
    SPMD graph processes 8 query tiles whose key ranges are padded by at most
    one 128-tile (+6% flops) and the pad/diagonal is handled by a host-
    provided additive mask over the last two key blocks of every slot.
  - the K projection is folded away algebraically: scores = q k^T =
    x_q (Wq^T Wk) x_kv^T, with M = Wq^T Wk precomputed on the host from the
    weights. The device computes qk = x_q @ M and contracts scores directly
    against x^T, which is already resident in SBUF. Similarly the V
    projection runs after the attention-weighted sum: out = (probs @ x) Wv^T.
  - the per-slot work is software-pipelined as
    ... scores(s+1) | out-projection(s) | attn-V(s+1) ... so the last slot's
    exp/copy latency hides under the previous slot's matmuls.
"""

import math
import os
import sys

import numpy as np

if not any(os.path.isdir(os.path.join(p, "concourse")) for p in sys.path):
    sys.path.insert(0, "/opt/trn_rl_repo")

import concourse.bass as bass  # noqa: E402
import concourse.mybir as mybir  # noqa: E402
from concourse import bacc, tile  # noqa: E402
from concourse.bass_utils import run_bass_kernel_spmd  # noqa: E402
from concourse.masks import make_identity  # noqa: E402

import ml_dtypes  # noqa: E402

B, S, D = 4, 2048, 768
P = 128
NT = S // P          # 16 key tiles per batch
DC = D // P          # 6 contraction chunks
NSLOT = 8            # query tiles per core
QROWS = NSLOT * P    # 1024 query rows per core
N_CORES = 8
SCALE = 1.0 / math.sqrt(D)

SIDE_A = [0, 3, 4, 7, 8, 11, 12, 15]   # causal lengths 1,4,5,8,9,12,13,16
SIDE_B = [1, 2, 5, 6, 9, 10, 13, 14]   # causal lengths 2,3,6,7,10,11,14,15
CAP = [2, 4, 6, 8, 10, 12, 14, 16]     # static key tiles per slot (>= real)

BF16 = ml_dtypes.bfloat16

_NC = None


def build():
    """Build + compile the single SPMD graph run by all 8 cores."""
    f32 = mybir.dt.float32
    bf16 = mybir.dt.bfloat16

    nc = bacc.Bacc("TRN2", target_bir_lowering=False, debug=False,
                   num_devices=N_CORES)

    # inputs come pre-packed as [P, chunk, width] (host layout transform)
    xq_d = nc.dram_tensor("xqT", [P, 2, DC, 512], bf16,
                          kind="ExternalInput").ap()
    xkv_d = nc.dram_tensor("xkvT", [P, DC, S], bf16,
                           kind="ExternalInput").ap()
    xkvr_d = nc.dram_tensor("xkvR", [P, NT, D], bf16,
                            kind="ExternalInput").ap()
    m_d = nc.dram_tensor("mT", [P, 3, DC, 256], bf16,
                         kind="ExternalInput").ap()
    wv_d = nc.dram_tensor("wvT", [P, DC, D], bf16, kind="ExternalInput").ap()
    mask_d = nc.dram_tensor("mask", [P, NSLOT, 256], bf16,
                            kind="ExternalInput").ap()
    out_d = nc.dram_tensor("out", [QROWS, D], f32, kind="ExternalOutput").ap()

    with tile.TileContext(nc) as tc:
        with (
            tc.tile_pool(name="const", bufs=1) as const,
            tc.tile_pool(name="probs", bufs=5) as probs_pool,
            tc.tile_pool(name="lsums", bufs=5) as lsum_pool,
            tc.tile_pool(name="pt", bufs=3) as pt_pool,
            tc.tile_pool(name="osb", bufs=2) as osb_pool,
            tc.tile_pool(name="small", bufs=2) as small,
            tc.tile_pool(name="ps_s", bufs=2, space="PSUM") as ps_s,
            tc.tile_pool(name="ps_tr", bufs=2, space="PSUM") as ps_tr,
            tc.tile_pool(name="ps_o", bufs=2, space="PSUM") as ps_o,
        ):
            # ---- persistent SBUF tensors, split in halves of 3 d-chunks
            # each so input DMA (12KB+ descriptors) overlaps the projections
            HC = DC // 2
            m_p = [const.tile([P, DC, 256], bf16, tag=f"mp{i}",
                              name=f"mp{i}") for i in range(3)]
            wv_h = [const.tile([P, HC, D], bf16, tag=f"wvh{h}", name=f"wvh{h}")
                    for h in range(2)]
            xq_g = [const.tile([P, DC, 512], bf16, tag=f"xqg{g}",
                               name=f"xqg{g}") for g in range(2)]
            xkv_h = [const.tile([P, HC, S], bf16, tag=f"xkvh{h}",
                                name=f"xkvh{h}") for h in range(2)]

            def chunk(tiles, dc):
                return tiles[dc // HC][:, dc % HC, :]

            wv_c = [chunk(wv_h, c) for c in range(DC)]
            xkv_c = [chunk(xkv_h, c) for c in range(DC)]
            mask_sb = const.tile([P, NSLOT, 256], bf16, tag="mask")
            ident = const.tile([P, P], bf16, tag="ident")
            qt_sb = const.tile([P, DC, QROWS], bf16, tag="qt")
            xv_h = [const.tile([P, NT // 2, D], bf16, tag=f"xvh{h}",
                                name=f"xvh{h}") for h in range(2)]

            # priority-ordered input DMAs: qk-projection operands first, then
            # the x^T chunks the score matmuls stream from
            nc.sync.dma_start(out=xq_g[0][:, :, :], in_=xq_d[:, 0, :, :])
            for i in range(3):
                nc.sync.dma_start(out=m_p[i][:, :, :], in_=m_d[:, i, :, :])
            nc.sync.dma_start(out=xkv_h[0][:, :, :], in_=xkv_d[:, 0:HC, :])
            nc.sync.dma_start(out=xq_g[1][:, :, :], in_=xq_d[:, 1, :, :])
            nc.sync.dma_start(out=xkv_h[1][:, :, :], in_=xkv_d[:, HC:DC, :])
            make_identity(nc, ident[:, :])

            # HAM warm-up: keep the PE busy while the first inputs stream in
            # so the real matmuls run at 2.4GHz from the start.
            warm = ps_tr.tile([P, P], f32, tag="tr", name="warm")
            for _ in range(40):
                nc.tensor.matmul(warm[:, :], ident[:, :], ident[:, :],
                                 start=True, stop=True)

            # ---- qkT[e,q] = (x_q @ M)^T projection (group-major: starts on
            # the first DMAs; group 0 covers slots 0-3)
            for g in range(QROWS // 512):
                for oc in range(DC):
                    ps = ps_s.tile([P, 512], f32, tag="mm512")
                    for dc in range(DC):
                        nc.tensor.matmul(
                            ps[:, :],
                            m_p[oc // 2][:, dc,
                                         (oc % 2) * P:(oc % 2 + 1) * P],
                            xq_g[g][:, dc, :],
                            start=(dc == 0), stop=(dc == DC - 1))
                    nc.scalar.copy(qt_sb[:, oc, g * 512:(g + 1) * 512],
                                   ps[:, :])
                if g == 0:
                    nc.sync.dma_start(out=mask_sb[:, :, :],
                                      in_=mask_d[:, :, :])
                    nc.sync.dma_start(
                        out=xv_h[0][:, :, :],
                        in_=xkvr_d[:, 0:NT // 2, :])
                    for h in range(2):
                        nc.sync.dma_start(out=wv_h[h][:, :, :],
                                          in_=wv_d[:, h * HC:(h + 1) * HC, :])
                    nc.sync.dma_start(
                        out=xv_h[1][:, :, :],
                        in_=xkvr_d[:, NT // 2:NT, :])

            # ---- attention, software-pipelined per 128-row query slot
            slot_bufs = {}

            def emit_scores(s):
                L = CAP[s]
                nk = L * P
                nkg = (nk + 511) // 512
                probs = probs_pool.tile([P, S], bf16, tag="probs",
                                        name=f"probs{s}")
                lsum = lsum_pool.tile([P, 4], f32, tag="lsum",
                                      name=f"lsum{s}")
                slot_bufs[s] = (probs, lsum)
                for g in range(nkg):
                    kw = min(512, nk - g * 512)
                    ps = ps_s.tile([P, 512], f32, tag="mm512",
                                   name=f"sps{s}_{g}")
                    for oc in range(DC):
                        nc.tensor.matmul(
                            ps[:, :kw],
                            qt_sb[:, oc, s * P:(s + 1) * P],
                            xkv_c[oc][:, g * 512:g * 512 + kw],
                            start=(oc == 0), stop=(oc == DC - 1))
                    mstart = (L - 2) * P   # masked region: last two blocks
                    if g * 512 <= mstart < g * 512 + kw:
                        off = mstart - g * 512
                        nc.vector.tensor_add(ps[:, off:off + 256],
                                             ps[:, off:off + 256],
                                             mask_sb[:, s, :])
                    nc.scalar.activation(
                        probs[:, g * 512:g * 512 + kw], ps[:, :kw],
                        mybir.ActivationFunctionType.Exp,
                        scale=SCALE, accum_out=lsum[:, g:g + 1])

            def emit_rest_av(s):
                """Transpose probability tiles and accumulate Y = probs @ x."""
                L = CAP[s]
                probs, lsum = slot_bufs[s]
                out_ps = ps_o.tile([P, D], f32, tag="mmout", name=f"ops{s}")
                slot_bufs[s] = (probs, lsum, out_ps)
                for kg in range((L + 3) // 4):     # 4 transposes per bank,
                    kn = min(4, L - kg * 4)        # one wide copy per group
                    tp = ps_tr.tile([P, 512], bf16, tag="tr", name=f"tp{s}")
                    for j in range(kn):
                        kt = kg * 4 + j
                        nc.tensor.transpose(tp[:, j * P:(j + 1) * P],
                                            probs[:, kt * P:(kt + 1) * P],
                                            ident[:, :])
                    pT = pt_pool.tile([P, 512], bf16, tag="pT", name=f"pT{s}")
                    nc.vector.tensor_copy(pT[:, 0:kn * P], tp[:, 0:kn * P])
                    for j in range(kn):
                        kt = kg * 4 + j
                        xv = xv_h[kt // (NT // 2)][:, kt % (NT // 2), :]
                        nc.tensor.matmul(out_ps[:, 0:512],
                                         pT[:, j * P:(j + 1) * P],
                                         xv[:, 0:512],
                                         start=(kt == 0), stop=(kt == L - 1))
                        nc.tensor.matmul(out_ps[:, 512:D],
                                         pT[:, j * P:(j + 1) * P],
                                         xv[:, 512:D],
                                         start=(kt == 0), stop=(kt == L - 1))

            def emit_rest_out(s):
                """Scale Y by 1/l, transpose, apply Wv: out = (Y/l) @ Wv^T."""
                L = CAP[s]
                nkg = (L * P + 511) // 512
                probs, lsum, out_ps = slot_bufs.pop(s)
                rinv = small.tile([P, 1], f32, tag="rinv", name=f"rinv{s}")
                if nkg > 1:
                    rsum = small.tile([P, 1], f32, tag="rsum",
                                      name=f"rsum{s}")
                    nc.vector.tensor_reduce(rsum[:, :], lsum[:, 0:nkg],
                                            axis=mybir.AxisListType.X,
                                            op=mybir.AluOpType.add)
                    nc.vector.reciprocal(rinv[:, :], rsum[:, :])
                else:
                    nc.vector.reciprocal(rinv[:, :], lsum[:, 0:1])

                y_lo = osb_pool.tile([P, 512], bf16, tag="ylo", name=f"ylo{s}")
                y_hi = osb_pool.tile([P, 256], bf16, tag="yhi", name=f"yhi{s}")
                nc.scalar.copy(y_lo[:, :], out_ps[:, 0:512])
                nc.vector.tensor_copy(y_hi[:, :], out_ps[:, 512:D])
                ytT = pt_pool.tile([P, D], bf16, tag="ytT", name=f"ytT{s}")
                for kg in range(2):
                    kn = 4 if kg == 0 else 2
                    tp = ps_tr.tile([P, 512], bf16, tag="tr", name=f"ytp{s}")
                    for j in range(kn):
                        dt = kg * 4 + j
                        ysrc = (y_lo[:, dt * P:(dt + 1) * P] if dt < 4 else
                                y_hi[:, (dt - 4) * P:(dt - 3) * P])
                        nc.tensor.transpose(tp[:, j * P:(j + 1) * P],
                                            ysrc, ident[:, :])
                    nc.vector.tensor_copy(
                        ytT[:, kg * 512:kg * 512 + kn * P],
                        tp[:, 0:kn * P])
                out2_ps = ps_o.tile([P, D], f32, tag="mmout", name=f"o2ps{s}")
                for dc in range(DC):
                    nc.tensor.matmul(out2_ps[:, 0:512],
                                     ytT[:, dc * P:(dc + 1) * P],
                                     wv_c[dc][:, 0:512],
                                     start=(dc == 0), stop=(dc == DC - 1))
                for dc in range(DC):
                    nc.tensor.matmul(out2_ps[:, 512:D],
                                     ytT[:, dc * P:(dc + 1) * P],
                                     wv_c[dc][:, 512:D],
                                     start=(dc == 0), stop=(dc == DC - 1))
                out_sb = osb_pool.tile([P, D], f32, tag="osb", name=f"osb{s}")
                if s == NSLOT - 1:
                    # last slot: nothing left to overlap -- scale on both
                    # engines (PSUM bank split at 512) to shorten the tail
                    nc.scalar.activation(out_sb[:, 0:512], out2_ps[:, 0:512],
                                         mybir.ActivationFunctionType.Copy,
                                         scale=rinv[:, :])
                    nc.sync.dma_start(out=out_d[s * P:(s + 1) * P, 0:512],
                                      in_=out_sb[:, 0:512])
                    nc.vector.tensor_scalar_mul(out_sb[:, 512:D],
                                                out2_ps[:, 512:D],
                                                rinv[:, :])
                    nc.sync.dma_start(out=out_d[s * P:(s + 1) * P, 512:D],
                                      in_=out_sb[:, 512:D])
                else:
                    for hcol in range(2):
                        cs = slice(hcol * 384, (hcol + 1) * 384)
                        nc.vector.tensor_scalar_mul(out_sb[:, cs],
                                                    out2_ps[:, cs],
                                                    rinv[:, :])
                        nc.sync.dma_start(out=out_d[s * P:(s + 1) * P, cs],
                                          in_=out_sb[:, cs])

            for s in range(NSLOT):
                emit_scores(s)
                emit_rest_av(s)
                emit_rest_out(s)

    nc.compile()
    return nc


def _pack(matT):
    """[D, W] (transposed operand) -> [P, DC, W] chunk layout, bf16."""
    d, w = matT.shape
    return np.ascontiguousarray(
        matT.reshape(d // P, P, w).transpose(1, 0, 2)).astype(BF16)


def shard_inputs(x, Wq, Wk, Wv):
    x = np.asarray(x, dtype=np.float32)
    # fold the K projection into the Q side: scores = x_q (Wq^T Wk) x_kv^T
    m = np.asarray(Wq, np.float32).T @ np.asarray(Wk, np.float32)
    mT = _pack(m)                                        # [P, DC, D]
    mT = np.ascontiguousarray(                           # [P, 3, DC, 256]
        mT.reshape(P, DC, 3, 256).transpose(0, 2, 1, 3))
    wvT = _pack(np.asarray(Wv, np.float32).T)
    in_maps = []
    for c in range(N_CORES):
        b, side = divmod(c, 2)
        qtiles = SIDE_A if side == 0 else SIDE_B
        xb = x[b]                                    # [S, D]
        xkvT = _pack(np.ascontiguousarray(xb.T))
        xkvR = np.ascontiguousarray(                     # [P, NT, D] row-major
            xb.astype(BF16).reshape(NT, P, D).transpose(1, 0, 2))
        xq = np.concatenate([xb[t * P:(t + 1) * P] for t in qtiles], axis=0)
        xqT = _pack(np.ascontiguousarray(xq.T))          # [P, DC, QROWS]
        xqT = np.ascontiguousarray(                      # [P, 2, DC, 512]
            xqT.reshape(P, DC, 2, 512).transpose(0, 2, 1, 3))
        mask = np.empty((NSLOT, P, 256), np.float32)
        for s, t in enumerate(qtiles):
            L = CAP[s]
            qidx = t * P + np.arange(P)[:, None]
            kidx = (L - 2) * P + np.arange(256)[None, :]
            mask[s] = np.where(kidx <= qidx, 0.0, -1e30).astype(np.float32)
        # mask dram layout [P, NSLOT, 256]
        mask = np.ascontiguousarray(mask.transpose(1, 0, 2)).astype(BF16)
        in_maps.append({"xqT": xqT, "xkvT": xkvT, "xkvR": xkvR, "mT": mT,
                        "wvT": wvT, "mask": mask})
    return in_maps


def unshard(results):
    out = np.empty((B, S, D), np.float32)
    for c in range(N_CORES):
        b, side = divmod(c, 2)
        qtiles = SIDE_A if side == 0 else SIDE_B
        oc = results[c]["out"]
        for s, t in enumerate(qtiles):
            out[b, t * P:(t + 1) * P] = oc[s * P:(s + 1) * P]
    return out


def run(inputs, trace=False, trace_cores=None):
    """Run on hardware; returns (output, BassKernelResults)."""
    global _NC
    if _NC is None:
        _NC = build()
    in_maps = shard_inputs(inputs["x"], inputs["Wq"], inputs["Wk"],
                           inputs["Wv"])
    res = run_bass_kernel_spmd(_NC, in_maps, core_ids=list(range(N_CORES)),
                               trace=trace, trace_cores=trace_cores)
    return unshard(res.results), res


def kernel(x, Wq, Wk, Wv):
    out, _ = run({"x": x, "Wq": Wq, "Wk": Wk, "Wv": Wv})
    return out


# revision 16
# speedup vs baseline: 1.0303x; 1.0049x over previous
"""Trainium2 Bass kernel for single-head causal attention.

Problem: x:[4,2048,768], Wq/Wk/Wv:[768,768] (torch-Linear layout, y = x @ W.T),
out = causal_softmax(q k^T / sqrt(768)) @ v, all float32.

Sharding (8 NeuronCores, no collectives):
  - core pair (2b, 2b+1) handles batch b.
  - per batch, the 16 query tiles of 128 rows are split between the pair as
    {0,3,4,7,8,11,12,15} and {1,2,5,6,9,10,13,14}. Sorted by causal length
    those are {1,4,5,8,9,12,13,16} and {2,3,6,7,10,11,14,15} key-tiles, so
    both sides fit the same static per-slot key budget {2,4,...,16}: the one
    SPMD graph processes 8 query tiles whose key ranges are padded by at most
    one 128-tile (+6% flops) and the pad/diagonal is handled by a host-
    provided additive mask over the last two key blocks of every slot.
  - the K projection is folded away algebraically: scores = q k^T =
    x_q (Wq^T Wk) x_kv^T, with M = Wq^T Wk precomputed on the host from the
    weights. The device computes qk = x_q @ M and contracts scores directly
    against x^T, which is already resident in SBUF. Similarly the V
    projection runs after the attention-weighted sum: out = (probs @ x) Wv^T.
  - host pre-transposes inputs (x^T, M^T-chunks), packs them into the SBUF
    chunk layout [128, chunk, width], and converts to bf16, so the device
    does no operand transposes; only the 128x128 probability tiles are
    transposed on the TensorEngine for the probs @ V matmul.
"""

import math
import os
import sys

import numpy as np

if not any(os.path.isdir(os.path.join(p, "concourse")) for p in sys.path):
    sys.path.insert(0, "/opt/trn_rl_repo")

import concourse.bass as bass  # noqa: E402
import concourse.mybir as mybir  # noqa: E402
from concourse import bacc, tile  # noqa: E402
from concourse.bass_utils import run_bass_kernel_spmd  # noqa: E402
from concourse.masks import make_identity  # noqa: E402

import ml_dtypes  # noqa: E402

B, S, D = 4, 2048, 768
P = 128
NT = S // P          # 16 key tiles per batch
DC = D // P          # 6 contraction chunks
NSLOT = 8            # query tiles per core
QROWS = NSLOT * P    # 1024 query rows per core
N_CORES = 8
SCALE = 1.0 / math.sqrt(D)

SIDE_A = [0, 3, 4, 7, 8, 11, 12, 15]   # causal lengths 1,4,5,8,9,12,13,16
SIDE_B = [1, 2, 5, 6, 9, 10, 13, 14]   # causal lengths 2,3,6,7,10,11,14,15
CAP = [2, 4, 6, 8, 10, 12, 14, 16]     # static key tiles per slot (>= real)

BF16 = ml_dtypes.bfloat16

_NC = None


def build():
    """Build + compile the single SPMD graph run by all 8 cores."""
    f32 = mybir.dt.float32
    bf16 = mybir.dt.bfloat16

    nc = bacc.Bacc("TRN2", target_bir_lowering=False, debug=False,
                   num_devices=N_CORES)

    # inputs come pre-packed as [P, chunk, width] (host layout transform)
    xq_d = nc.dram_tensor("xqT", [P, 2, DC, 512], bf16,
                          kind="ExternalInput").ap()
    xkv_d = nc.dram_tensor("xkvT", [P, DC, S], bf16,
                           kind="ExternalInput").ap()
    xkvr_d = nc.dram_tensor("xkvR", [P, NT, D], bf16,
                            kind="ExternalInput").ap()
    m_d = nc.dram_tensor("mT", [P, 3, DC, 256], bf16,
                         kind="ExternalInput").ap()
    wv_d = nc.dram_tensor("wvT", [P, DC, D], bf16, kind="ExternalInput").ap()
    mask_d = nc.dram_tensor("mask", [P, NSLOT, 256], bf16,
                            kind="ExternalInput").ap()
    out_d = nc.dram_tensor("out", [QROWS, D], f32, kind="ExternalOutput").ap()

    with tile.TileContext(nc) as tc:
        with (
            tc.tile_pool(name="const", bufs=1) as const,
            tc.tile_pool(name="probs", bufs=5) as probs_pool,
            tc.tile_pool(name="lsums", bufs=5) as lsum_pool,
            tc.tile_pool(name="pt", bufs=3) as pt_pool,
            tc.tile_pool(name="osb", bufs=2) as osb_pool,
            tc.tile_pool(name="small", bufs=2) as small,
            tc.tile_pool(name="ps_s", bufs=2, space="PSUM") as ps_s,
            tc.tile_pool(name="ps_tr", bufs=2, space="PSUM") as ps_tr,
            tc.tile_pool(name="ps_o", bufs=2, space="PSUM") as ps_o,
        ):
            # ---- persistent SBUF tensors, split in halves of 3 d-chunks
            # each so input DMA (12KB+ descriptors) overlaps the projections
            HC = DC // 2
            m_p = [const.tile([P, DC, 256], bf16, tag=f"mp{i}",
                              name=f"mp{i}") for i in range(3)]
            wv_h = [const.tile([P, HC, D], bf16, tag=f"wvh{h}", name=f"wvh{h}")
                    for h in range(2)]
            xq_g = [const.tile([P, DC, 512], bf16, tag=f"xqg{g}",
                               name=f"xqg{g}") for g in range(2)]
            xkv_h = [const.tile([P, HC, S], bf16, tag=f"xkvh{h}",
                                name=f"xkvh{h}") for h in range(2)]

            def chunk(tiles, dc):
                return tiles[dc // HC][:, dc % HC, :]

            wv_c = [chunk(wv_h, c) for c in range(DC)]
            xkv_c = [chunk(xkv_h, c) for c in range(DC)]
            mask_sb = const.tile([P, NSLOT, 256], bf16, tag="mask")
            ident = const.tile([P, P], bf16, tag="ident")
            qt_sb = const.tile([P, DC, QROWS], bf16, tag="qt")
            xv_h = [const.tile([P, NT // 2, D], bf16, tag=f"xvh{h}",
                                name=f"xvh{h}") for h in range(2)]

            # priority-ordered input DMAs: qk-projection operands first, then
            # the x^T chunks the score matmuls stream from
            nc.sync.dma_start(out=xq_g[0][:, :, :], in_=xq_d[:, 0, :, :])
            for i in range(3):
                nc.sync.dma_start(out=m_p[i][:, :, :], in_=m_d[:, i, :, :])
            nc.sync.dma_start(out=xkv_h[0][:, :, :], in_=xkv_d[:, 0:HC, :])
            nc.sync.dma_start(out=xq_g[1][:, :, :], in_=xq_d[:, 1, :, :])
            nc.sync.dma_start(out=xkv_h[1][:, :, :], in_=xkv_d[:, HC:DC, :])
            make_identity(nc, ident[:, :])

            # HAM warm-up: keep the PE busy while the first inputs stream in
            # so the real matmuls run at 2.4GHz from the start.
            warm = ps_tr.tile([P, P], f32, tag="tr", name="warm")
            for _ in range(40):
                nc.tensor.matmul(warm[:, :], ident[:, :], ident[:, :],
                                 start=True, stop=True)

            # ---- qkT[e,q] = (x_q @ M)^T projection (group-major: starts on
            # the first DMAs; group 0 covers slots 0-3)
            for g in range(QROWS // 512):
                for oc in range(DC):
                    ps = ps_s.tile([P, 512], f32, tag="mm512")
                    for dc in range(DC):
                        nc.tensor.matmul(
                            ps[:, :],
                            m_p[oc // 2][:, dc,
                                         (oc % 2) * P:(oc % 2 + 1) * P],
                            xq_g[g][:, dc, :],
                            start=(dc == 0), stop=(dc == DC - 1))
                    nc.scalar.copy(qt_sb[:, oc, g * 512:(g + 1) * 512],
                                   ps[:, :])
                if g == 0:
                    nc.sync.dma_start(out=mask_sb[:, :, :],
                                      in_=mask_d[:, :, :])
                    nc.sync.dma_start(
                        out=xv_h[0][:, :, :],
                        in_=xkvr_d[:, 0:NT // 2, :])
                    for h in range(2):
                        nc.sync.dma_start(out=wv_h[h][:, :, :],
                                          in_=wv_d[:, h * HC:(h + 1) * HC, :])
                    nc.sync.dma_start(
                        out=xv_h[1][:, :, :],
                        in_=xkvr_d[:, NT // 2:NT, :])

            # ---- attention, software-pipelined per 128-row query slot
            slot_bufs = {}

            def emit_scores(s):
                L = CAP[s]
                nk = L * P
                nkg = (nk + 511) // 512
                probs = probs_pool.tile([P, S], bf16, tag="probs",
                                        name=f"probs{s}")
                lsum = lsum_pool.tile([P, 4], f32, tag="lsum",
                                      name=f"lsum{s}")
                slot_bufs[s] = (probs, lsum)
                for g in range(nkg):
                    kw = min(512, nk - g * 512)
                    ps = ps_s.tile([P, 512], f32, tag="mm512",
                                   name=f"sps{s}_{g}")
                    for oc in range(DC):
                        nc.tensor.matmul(
                            ps[:, :kw],
                            qt_sb[:, oc, s * P:(s + 1) * P],
                            xkv_c[oc][:, g * 512:g * 512 + kw],
                            start=(oc == 0), stop=(oc == DC - 1))
                    mstart = (L - 2) * P   # masked region: last two blocks
                    if g * 512 <= mstart < g * 512 + kw:
                        off = mstart - g * 512
                        nc.vector.tensor_add(ps[:, off:off + 256],
                                             ps[:, off:off + 256],
                                             mask_sb[:, s, :])
                    nc.scalar.activation(
                        probs[:, g * 512:g * 512 + kw], ps[:, :kw],
                        mybir.ActivationFunctionType.Exp,
                        scale=SCALE, accum_out=lsum[:, g:g + 1])

            def emit_rest_av(s):
                """Transpose probability tiles and accumulate Y = probs @ x."""
                L = CAP[s]
                probs, lsum = slot_bufs[s]
                out_ps = ps_o.tile([P, D], f32, tag="mmout", name=f"ops{s}")
                slot_bufs[s] = (probs, lsum, out_ps)
                for kg in range((L + 3) // 4):     # 4 transposes per bank,
                    kn = min(4, L - kg * 4)        # one wide copy per group
                    tp = ps_tr.tile([P, 512], bf16, tag="tr", name=f"tp{s}")
                    for j in range(kn):
                        kt = kg * 4 + j
                        nc.tensor.transpose(tp[:, j * P:(j + 1) * P],
                                            probs[:, kt * P:(kt + 1) * P],
                                            ident[:, :])
                    pT = pt_pool.tile([P, 512], bf16, tag="pT", name=f"pT{s}")
                    nc.vector.tensor_copy(pT[:, 0:kn * P], tp[:, 0:kn * P])
                    for j in range(kn):
                        kt = kg * 4 + j
                        xv = xv_h[kt // (NT // 2)][:, kt % (NT // 2), :]
                        nc.tensor.matmul(out_ps[:, 0:512],
                                         pT[:, j * P:(j + 1) * P],
                                         xv[:, 0:512],
                                         start=(kt == 0), stop=(kt == L - 1))
                        nc.tensor.matmul(out_ps[:, 512:D],
                                         pT[:, j * P:(j + 1) * P],
                                         xv[:, 512:D],
                                         start=(kt == 0), stop=(kt == L - 1))

            def emit_rest_out(s):
                """Scale Y by 1/l, transpose, apply Wv: out = (Y/l) @ Wv^T."""
                L = CAP[s]
                nkg = (L * P + 511) // 512
                probs, lsum, out_ps = slot_bufs.pop(s)
                rinv = small.tile([P, 1], f32, tag="rinv", name=f"rinv{s}")
                if nkg > 1:
                    rsum = small.tile([P, 1], f32, tag="rsum",
                                      name=f"rsum{s}")
                    nc.vector.tensor_reduce(rsum[:, :], lsum[:, 0:nkg],
                                            axis=mybir.AxisListType.X,
                                            op=mybir.AluOpType.add)
                    nc.vector.reciprocal(rinv[:, :], rsum[:, :])
                else:
                    nc.vector.reciprocal(rinv[:, :], lsum[:, 0:1])

                y_lo = osb_pool.tile([P, 512], bf16, tag="ylo", name=f"ylo{s}")
                y_hi = osb_pool.tile([P, 256], bf16, tag="yhi", name=f"yhi{s}")
                nc.scalar.copy(y_lo[:, :], out_ps[:, 0:512])
                nc.vector.tensor_copy(y_hi[:, :], out_ps[:, 512:D])
                ytT = pt_pool.tile([P, D], bf16, tag="ytT", name=f"ytT{s}")
                for kg in range(2):
                    kn = 4 if kg == 0 else 2
                    tp = ps_tr.tile([P, 512], bf16, tag="tr", name=f"ytp{s}")
                    for j in range(kn):
                        dt = kg * 4 + j
                        ysrc = (y_lo[:, dt * P:(dt + 1) * P] if dt < 4 else
                                y_hi[:, (dt - 4) * P:(dt - 3) * P])
                        nc.tensor.transpose(tp[:, j * P:(j + 1) * P],
                                            ysrc, ident[:, :])
                    nc.vector.tensor_copy(
                        ytT[:, kg * 512:kg * 512 + kn * P],
                        tp[:, 0:kn * P])
                out2_ps = ps_o.tile([P, D], f32, tag="mmout", name=f"o2ps{s}")
                for dc in range(DC):
                    nc.tensor.matmul(out2_ps[:, 0:512],
                                     ytT[:, dc * P:(dc + 1) * P],
                                     wv_c[dc][:, 0:512],
                                     start=(dc == 0), stop=(dc == DC - 1))
                for dc in range(DC):
                    nc.tensor.matmul(out2_ps[:, 512:D],
                                     ytT[:, dc * P:(dc + 1) * P],
                                     wv_c[dc][:, 512:D],
                                     start=(dc == 0), stop=(dc == DC - 1))
                out_sb = osb_pool.tile([P, D], f32, tag="osb", name=f"osb{s}")
                if s == NSLOT - 1:
                    # last slot: nothing left to overlap -- scale on both
                    # engines (PSUM bank split at 512) to shorten the tail
                    nc.scalar.activation(out_sb[:, 0:512], out2_ps[:, 0:512],
                                         mybir.ActivationFunctionType.Copy,
                                         scale=rinv[:, :])
                    nc.sync.dma_start(out=out_d[s * P:(s + 1) * P, 0:512],
                                      in_=out_sb[:, 0:512])
                    nc.vector.tensor_scalar_mul(out_sb[:, 512:D],
                                                out2_ps[:, 512:D],
                                                rinv[:, :])
                    nc.sync.dma_start(out=out_d[s * P:(s + 1) * P, 512:D],
                                      in_=out_sb[:, 512:D])
                else:
                    for hcol in range(2):
                        cs = slice(hcol * 384, (hcol + 1) * 384)
                        nc.vector.tensor_scalar_mul(out_sb[:, cs],
                                                    out2_ps[:, cs],
                                                    rinv[:, :])
                        nc.sync.dma_start(out=out_d[s * P:(s + 1) * P, cs],
                                          in_=out_sb[:, cs])

            for s in range(NSLOT):
                emit_scores(s)
                emit_rest_av(s)
                emit_rest_out(s)

    nc.compile()
    return nc


def _pack(matT):
    """[D, W] (transposed operand) -> [P, DC, W] chunk layout, bf16."""
    d, w = matT.shape
    return np.ascontiguousarray(
        matT.reshape(d // P, P, w).transpose(1, 0, 2)).astype(BF16)


def shard_inputs(x, Wq, Wk, Wv):
    x = np.asarray(x, dtype=np.float32)
    # fold the K projection into the Q side: scores = x_q (Wq^T Wk) x_kv^T
    m = np.asarray(Wq, np.float32).T @ np.asarray(Wk, np.float32)
    mT = _pack(m)                                        # [P, DC, D]
    mT = np.ascontiguousarray(                           # [P, 3, DC, 256]
        mT.reshape(P, DC, 3, 256).transpose(0, 2, 1, 3))
    wvT = _pack(np.asarray(Wv, np.float32).T)
    in_maps = []
    for c in range(N_CORES):
        b, side = divmod(c, 2)
        qtiles = SIDE_A if side == 0 else SIDE_B
        xb = x[b]                                    # [S, D]
        xkvT = _pack(np.ascontiguousarray(xb.T))
        xkvR = np.ascontiguousarray(                     # [P, NT, D] row-major
            xb.astype(BF16).reshape(NT, P, D).transpose(1, 0, 2))
        xq = np.concatenate([xb[t * P:(t + 1) * P] for t in qtiles], axis=0)
        xqT = _pack(np.ascontiguousarray(xq.T))          # [P, DC, QROWS]
        xqT = np.ascontiguousarray(                      # [P, 2, DC, 512]
            xqT.reshape(P, DC, 2, 512).transpose(0, 2, 1, 3))
        mask = np.empty((NSLOT, P, 256), np.float32)
        for s, t in enumerate(qtiles):
            L = CAP[s]
            qidx = t * P + np.arange(P)[:, None]
            kidx = (L - 2) * P + np.arange(256)[None, :]
            mask[s] = np.where(kidx <= qidx, 0.0, -1e30).astype(np.float32)
        # mask dram layout [P, NSLOT, 256]
        mask = np.ascontiguousarray(mask.transpose(1, 0, 2)).astype(BF16)
        in_maps.append({"xqT": xqT, "xkvT": xkvT, "xkvR": xkvR, "mT": mT,
                        "wvT": wvT, "mask": mask})
    return in_maps


def unshard(results):
    out = np.empty((B, S, D), np.float32)
    for c in range(N_CORES):
        b, side = divmod(c, 2)
        qtiles = SIDE_A if side == 0 else SIDE_B
        oc = results[c]["out"]
        for s, t in enumerate(qtiles):
            out[b, t * P:(t + 1) * P] = oc[s * P:(s + 1) * P]
    return out


def run(inputs, trace=False, trace_cores=None):
    """Run on hardware; returns (output, BassKernelResults)."""
    global _NC
    if _NC is None:
        _NC = build()
    in_maps = shard_inputs(inputs["x"], inputs["Wq"], inputs["Wk"],
                           inputs["Wv"])
    res = run_bass_kernel_spmd(_NC, in_maps, core_ids=list(range(N_CORES)),
                               trace=trace, trace_cores=trace_cores)
    return unshard(res.results), res


def kernel(x, Wq, Wk, Wv):
    out, _ = run({"x": x, "Wq": Wq, "Wk": Wk, "Wv": Wv})
    return out
